# revision 10
# baseline (speedup 1.0000x reference)
"""Trainium2 Bass kernel for flax MultiHeadDotProductAttention.

Shapes (hardcoded): B=4, Q=K=1500, D=1024, H=16, HD=64.
Sharding: 8 cores = 4 batches x 2 head-groups (8 heads each).
Each core computes its batch's attention output for its 8 heads plus the
output projection restricted to those heads; the host sums the two
head-group partials per batch and adds bo.

Dataflow per core (all layouts chosen so no on-device transposes are
needed; host passes x pre-transposed):
  qT/kT [hhd, seq] and v [seq, hhd] via f32r projections;
  S^T[k,q] = kT.T-slices @ qT (K=64, row-packed 2 heads per PE slot);
  P^T = exp(S^T/8) on ScalarE (psum->sbuf, bf16);
  attn_outT += v_tile.T @ P^T (bf16, col-packed 2 heads per slot) and
  denominators via ones-vector matmuls accumulated over k tiles;
  normalization folded into the psum->sbuf copy; out-projection consumes
  the normalized [hhd, q] tiles as stationary operands -> natural [q, d]
  output tiles DMA'd straight to HBM.
"""

import os
import sys

sys.path.insert(0, "/opt/trn_rl_repo")

import numpy as np  # noqa: E402
import concourse.bacc as bacc  # noqa: E402
import concourse.mybir as mybir  # noqa: E402
import concourse.tile as tile  # noqa: E402
from concourse.bass_utils import run_bass_kernel_spmd  # noqa: E402

F32 = mybir.dt.float32
F32R = mybir.dt.float32r
BF16 = mybir.dt.bfloat16
AF = mybir.ActivationFunctionType

B, SEQ, D, H, HD = 4, 1500, 1024, 16, 64
HG = 8                      # heads per group
HHD = HG * HD               # 512
DCH = D // 128              # 8 d-chunks
HB = HHD // 128             # 4 hhd blocks (2 heads each)
NPAIR = HB                  # 4 head pairs per group
QC = [(0, 512), (512, 512), (1024, 476)]          # q chunks
KT = [(i * 128, min(128, SEQ - i * 128)) for i in range((SEQ + 127) // 128)]
NKT = len(KT)               # 12 (last tile 92 rows)


def _build():
    nc = bacc.Bacc("TRN2", target_bir_lowering=False, debug=False, num_devices=8)

    xqT = nc.declare_dram_parameter("xqT", [D, SEQ], F32, isOutput=False)
    xkvT = nc.declare_dram_parameter("xkvT", [D, SEQ], F32, isOutput=False)
    wq_d = nc.declare_dram_parameter("wq", [D, HHD], F32, isOutput=False)
    wk_d = nc.declare_dram_parameter("wk", [D, HHD], F32, isOutput=False)
    wv_d = nc.declare_dram_parameter("wv", [D, HHD], F32, isOutput=False)
    wo_d = nc.declare_dram_parameter("wo", [HHD, D], F32, isOutput=False)
    bq_d = nc.declare_dram_parameter("bq", [1, HHD], F32, isOutput=False)
    bk_d = nc.declare_dram_parameter("bk", [1, HHD], F32, isOutput=False)
    bv_d = nc.declare_dram_parameter("bv", [1, HHD], F32, isOutput=False)
    sel_d = nc.declare_dram_parameter("sel", [64, 128], F32, isOutput=False)
    ones_d = nc.declare_dram_parameter("ones1", [1, 512], F32, isOutput=False)
    zr_d = nc.declare_dram_parameter("zr", [64, 512], F32, isOutput=False)
    out_d = nc.declare_dram_parameter("out", [SEQ, D], F32, isOutput=True)

    with tile.TileContext(nc) as tc:
        from contextlib import ExitStack

        with ExitStack() as ctx:
            ctx.enter_context(nc.allow_low_precision(
                reason="f32r/bf16 matmul operands; psum accumulation is fp32"
            ))
            const = ctx.enter_context(tc.tile_pool(name="const", bufs=1))
            ones_r = const.tile([1, 512], F32R, tag="ones")
            nc.sync.dma_start(ones_r[:], ones_d[:].bitcast(F32R))
            dones = const.tile([128, 1], BF16, tag="dones")
            nc.vector.memset(dones[:], 1.0)
            sel_sb = const.tile([64, 128], F32R, tag="sel")
            nc.sync.dma_start(sel_sb[:], sel_d[:].bitcast(F32R))
            rr = const.tile([64, 512], F32R, tag="rr")
            nc.sync.dma_start(rr[:], zr_d[:].bitcast(F32R))
            bq_sb = const.tile([1, HHD], F32R, tag="bq")
            nc.sync.dma_start(bq_sb[:], bq_d[:].bitcast(F32R))
            bk_sb = const.tile([1, HHD], F32R, tag="bk")
            nc.sync.dma_start(bk_sb[:], bk_d[:].bitcast(F32R))
            bv_sb = const.tile([1, HHD], F32R, tag="bv")
            nc.sync.dma_start(bv_sb[:], bv_d[:].bitcast(F32R))

            # persistent activations for the attention phase
            qT = const.tile([128, HB, SEQ], F32R, tag="qT")    # [hhd%128, blk, q]
            kT = const.tile([128, HB, SEQ], F32R, tag="kT")
            v_sb = const.tile([128, NKT, HHD], BF16, tag="v")  # [k%128, ktile, hhd]

            wpool = ctx.enter_context(tc.tile_pool(name="w", bufs=2))
            mm_ps = ctx.enter_context(
                tc.tile_pool(name="mmps", bufs=2, space="PSUM")
            )

            def load_w(dram, cols):
                t = wpool.tile([128, D // 128, cols], F32R, tag="w", bufs=2)
                nc.sync.dma_start(
                    t[:], dram.rearrange("(c p) n -> p c n", p=128).bitcast(F32R)
                )
                return t

            # ---------------- phase 1: projections ----------------
            with tc.tile_pool(name="x", bufs=8) as xpool:

                def load_x(dram):
                    xs = []
                    for c in range(DCH):
                        t = xpool.tile([128, SEQ], F32R, tag="xc", bufs=8)
                        nc.sync.dma_start(
                            t[:], dram[c * 128:(c + 1) * 128, :].bitcast(F32R)
                        )
                        xs.append(t)
                    return xs

                def proj_T(dst, w_sb, b_sb, xs):
                    # dst[:, hb, q] = (x @ W + b)^T rows hb*128..+128
                    for hb in range(HB):
                        for (qo, cw) in QC:
                            ps = mm_ps.tile([128, 512], F32, tag="mm", bufs=2)
                            for c in range(DCH):
                                nc.tensor.matmul(
                                    ps[:, :cw],
                                    w_sb[:, c, hb * 128:(hb + 1) * 128],
                                    xs[c][:, qo:qo + cw],
                                    start=(c == 0), stop=False,
                                )
                            nc.tensor.matmul(
                                ps[:, :cw],
                                b_sb[0:1, hb * 128:(hb + 1) * 128],
                                ones_r[0:1, :cw],
                                start=False, stop=True,
                            )
                            nc.vector.tensor_copy(
                                dst[:, hb, qo:qo + cw], ps[:, :cw]
                            )

                def proj_v(dst, w_sb, b_sb, xs):
                    # dst[:, kt, hhd] = (x @ W + b) rows kt*128..
                    for kt, (ko, kh) in enumerate(KT):
                        ps = mm_ps.tile([128, 512], F32, tag="mm", bufs=2)
                        for c in range(DCH):
                            nc.tensor.matmul(
                                ps[:kh, :],
                                xs[c][:, ko:ko + kh],
                                w_sb[:, c, :],
                                start=(c == 0), stop=False,
                            )
                        nc.tensor.matmul(
                            ps[:kh, :],
                            ones_r[0:1, :kh],
                            b_sb[0:1, :],
                            start=False, stop=True,
                        )
                        nc.vector.tensor_copy(dst[:kh, kt, :], ps[:kh, :])

                wk_sb = load_w(wk_d, HHD)
                wv_sb = load_w(wv_d, HHD)
                xkv = load_x(xkvT)
                proj_T(kT, wk_sb, bk_sb, xkv)
                proj_v(v_sb, wv_sb, bv_sb, xkv)
                wq_sb = load_w(wq_d, HHD)
                xq = load_x(xqT)
                proj_T(qT, wq_sb, bq_sb, xq)

            wo_sb = wpool.tile([128, HB, D], F32R, tag="w", bufs=2)
            nc.sync.dma_start(
                wo_sb[:], wo_d.rearrange("(c p) n -> p c n", p=128).bitcast(F32R)
            )

            # ---------------- phase 2+3: attention + out-proj ----------------
            st_ps = ctx.enter_context(tc.tile_pool(name="stps", bufs=2, space="PSUM"))
            pair_ps = ctx.enter_context(tc.tile_pool(name="pairps", bufs=1, space="PSUM"))
            den_ps = ctx.enter_context(tc.tile_pool(name="denps", bufs=1, space="PSUM"))
            p_pool = ctx.enter_context(tc.tile_pool(name="p", bufs=4))
            an_pool = ctx.enter_context(tc.tile_pool(name="an", bufs=8))
            small = ctx.enter_context(tc.tile_pool(name="small", bufs=4))

            for (qo, cw) in QC:
                anorms = []
                for j in range(NPAIR):
                    den = den_ps.tile([128, 512], F32, tag="den", bufs=1)
                    pair = pair_ps.tile([128, 512], F32, tag="pair", bufs=1)
                    for kt, (ko, kh) in enumerate(KT):
                        st = st_ps.tile([128, 2, 512], F32, tag="st", bufs=2)
                        # S^T for the even/odd head of pair j (row-packed)
                        nc.tensor.matmul(
                            st[:kh, 0, :cw],
                            kT[0:64, j, ko:ko + kh],
                            qT[0:64, j, qo:qo + cw],
                            start=True, stop=True,
                        )
                        nc.tensor.matmul(
                            st[:kh, 1, :cw],
                            kT[64:128, j, ko:ko + kh],
                            qT[64:128, j, qo:qo + cw],
                            start=True, stop=True,
                        )
                        p = p_pool.tile([128, 2, 512], BF16, tag="p", bufs=4)
                        nc.scalar.activation(
                            p[:kh, :, :cw], st[:kh, :, :cw], AF.Exp,
                            scale=0.125,
                        )
                        # attn @ V (bf16, col-packed pair into one psum tile)
                        nc.tensor.matmul(
                            pair[0:64, :cw],
                            v_sb[0:kh, kt, (2 * j) * 64:(2 * j) * 64 + 64],
                            p[0:kh, 0, :cw],
                            start=(kt == 0), stop=(kt == NKT - 1),
                            skip_group_check=True,
                        )
                        nc.tensor.matmul(
                            pair[64:128, :cw],
                            v_sb[0:kh, kt, (2 * j + 1) * 64:(2 * j + 1) * 64 + 64],
                            p[0:kh, 1, :cw],
                            start=(kt == 0), stop=(kt == NKT - 1),
                            skip_group_check=True,
                        )
                        # softmax denominators (M=1 matmuls, col-packed)
                        nc.tensor.matmul(
                            den[0:1, :cw],
                            dones[0:kh, 0:1],
                            p[0:kh, 0, :cw],
                            start=(kt == 0), stop=(kt == NKT - 1),
                            tile_position=(0, 0), skip_group_check=True,
                        )
                        nc.tensor.matmul(
                            den[32:33, :cw],
                            dones[0:kh, 0:1],
                            p[0:kh, 1, :cw],
                            start=(kt == 0), stop=(kt == NKT - 1),
                            tile_position=(0, 32), skip_group_check=True,
                        )

                    # normalize: recip of the 2 denominator rows, broadcast
                    # across partitions via a tiny selector matmul
                    nc.vector.reciprocal(rr[0:1, :cw], den[0:1, :cw])
                    nc.vector.reciprocal(rr[32:33, :cw], den[32:33, :cw])
                    rb_ps = mm_ps.tile([128, 512], F32, tag="mm", bufs=2)
                    nc.tensor.matmul(
                        rb_ps[:, :cw],
                        sel_sb[0:64, 0:128],
                        rr[0:64, :cw],
                        start=True, stop=True,
                    )
                    rb_sb = small.tile([128, 512], F32R, tag="rb", bufs=2)
                    nc.vector.tensor_copy(rb_sb[:, :cw], rb_ps[:, :cw])
                    an = an_pool.tile([128, 512], F32R, tag="an", bufs=8)
                    nc.vector.tensor_mul(
                        an[:, :cw], pair[:, :cw], rb_sb[:, :cw]
                    )
                    anorms.append(an)

                # out-projection for this q chunk (natural [q, d] layout)
                nsub = (cw + 127) // 128
                for s in range(nsub):
                    sw = min(128, cw - s * 128)
                    for dc in range(2):
                        op = mm_ps.tile([128, 512], F32, tag="mm", bufs=2)
                        for j in range(NPAIR):
                            nc.tensor.matmul(
                                op[:sw, :],
                                anorms[j][:, s * 128:s * 128 + sw],
                                wo_sb[:, j, dc * 512:(dc + 1) * 512],
                                start=(j == 0), stop=(j == NPAIR - 1),
                            )
                        osb = small.tile([128, 512], F32, tag="os", bufs=3)
                        nc.vector.tensor_copy(osb[:sw, :], op[:sw, :])
                        nc.sync.dma_start(
                            out_d[qo + s * 128:qo + s * 128 + sw,
                                  dc * 512:(dc + 1) * 512],
                            osb[:sw, :],
                        )

    nc.compile()
    return nc


_NC = None


def _get_nc():
    global _NC
    if _NC is None:
        _NC = _build()
    return _NC


def _sel_const():
    # sel[r, m] routes reciprocal row r to output partitions m:
    # row 0 (even head) -> partitions 0..63, row 32 (odd) -> 64..127
    sel = np.zeros((64, 128), np.float32)
    sel[0, 0:64] = 1.0
    sel[32, 64:128] = 1.0
    return sel


def _shard_inputs(inputs_q, inputs_kv, Wq, bq, Wk, bk, Wv, bv, Wo, bo):
    sel = _sel_const()
    ones1 = np.ones((1, 512), np.float32)
    zr = np.zeros((64, 512), np.float32)
    in_maps = []
    for b in range(B):
        xqT = np.ascontiguousarray(inputs_q[b].T).astype(np.float32)
        xkvT = np.ascontiguousarray(inputs_kv[b].T).astype(np.float32)
        for g in range(2):
            hs = slice(g * HG, (g + 1) * HG)
            in_maps.append({
                "xqT": xqT,
                "xkvT": xkvT,
                "wq": np.ascontiguousarray(Wq[:, hs, :].reshape(D, HHD), np.float32),
                "wk": np.ascontiguousarray(Wk[:, hs, :].reshape(D, HHD), np.float32),
                "wv": np.ascontiguousarray(Wv[:, hs, :].reshape(D, HHD), np.float32),
                "wo": np.ascontiguousarray(Wo[hs].reshape(HHD, D), np.float32),
                "bq": np.ascontiguousarray(bq[hs].reshape(1, HHD), np.float32),
                "bk": np.ascontiguousarray(bk[hs].reshape(1, HHD), np.float32),
                "bv": np.ascontiguousarray(bv[hs].reshape(1, HHD), np.float32),
                "sel": sel,
                "ones1": ones1,
                "zr": zr,
            })
    return in_maps


def _run(inputs, trace=False, trace_kwargs=None):
    nc = _get_nc()
    in_maps = _shard_inputs(**{k: np.asarray(v) for k, v in inputs.items()})
    res = run_bass_kernel_spmd(
        nc, in_maps, core_ids=list(range(2 * B)), trace=trace,
        **(trace_kwargs or {}),
    )
    bo = np.asarray(inputs["bo"], np.float32)
    out = np.empty((B, SEQ, D), np.float32)
    for b in range(B):
        out[b] = res.results[2 * b]["out"] + res.results[2 * b + 1]["out"] + bo
    return out, res


def kernel(**inputs):
    out, _ = _run(inputs, trace=False)
    return out


# revision 12
# speedup vs baseline: 1.1319x; 1.1319x over previous
"""Trainium2 Bass kernel for flax MultiHeadDotProductAttention.

Shapes (hardcoded): B=4, Q=K=1500, D=1024, H=16, HD=64.
Sharding: 8 cores = 4 batches x 2 head-groups (8 heads each).
Each core computes its batch's attention output for its 8 heads plus the
output projection restricted to those heads; the host sums the two
head-group partials per batch and adds bo.

Dataflow per core (all layouts chosen so no on-device transposes are
needed; host passes x pre-transposed):
  qT/kT [hhd, seq] and v [seq, hhd] via projection matmuls;
  S^T[k,q] = kT.T-slices @ qT (K=64, row-packed 2 heads per PE slot);
  P^T = exp(S^T/8) on ScalarE (psum->sbuf, bf16);
  attn_outT += v_tile.T @ P^T (bf16, col-packed 2 heads per slot) and
  denominators via ones-vector matmuls accumulated over k tiles;
  normalization folded into the psum->sbuf copy; out-projection consumes
  the normalized [hhd, q] tiles as stationary operands -> natural [q, d]
  output tiles DMA'd straight to HBM.

MODE: "bf16" (default) runs all big matmuls in bf16 (weight loads
overlap in-flight matmuls); "mixed" keeps projections/S^T/out-proj in
fp32r (higher precision, but each matmul pays a serialized weight load).
"""

import os
import sys

sys.path.insert(0, "/opt/trn_rl_repo")

import numpy as np  # noqa: E402
import ml_dtypes  # noqa: E402
import concourse.bacc as bacc  # noqa: E402
import concourse.mybir as mybir  # noqa: E402
import concourse.tile as tile  # noqa: E402
from concourse.bass_utils import run_bass_kernel_spmd  # noqa: E402

F32 = mybir.dt.float32
F32R = mybir.dt.float32r
BF16 = mybir.dt.bfloat16
AF = mybir.ActivationFunctionType

B, SEQ, D, H, HD = 4, 1500, 1024, 16, 64
HG = 8                      # heads per group
HHD = HG * HD               # 512
DCH = D // 128              # 8 d-chunks
HB = HHD // 128             # 4 hhd blocks (2 heads each)
NPAIR = HB                  # 4 head pairs per group
QC = [(0, 512), (512, 512), (1024, 476)]          # q chunks
KT = [(i * 128, min(128, SEQ - i * 128)) for i in range((SEQ + 127) // 128)]
NKT = len(KT)               # 12 (last tile 92 rows)

MODE = os.environ.get("BASS_MM_DTYPE", "bf16")


def _build(mode):
    mt = BF16 if mode == "bf16" else F32R          # big-matmul operand dtype
    hdt = F32 if mode == "bf16" else F32           # dram decl dtype handled below
    MTD = BF16 if mode == "bf16" else F32          # dram dtype for x/w/b

    nc = bacc.Bacc("TRN2", target_bir_lowering=False, debug=False, num_devices=8)

    xqT = nc.declare_dram_parameter("xqT", [D, SEQ], MTD, isOutput=False)
    xkvT = nc.declare_dram_parameter("xkvT", [D, SEQ], MTD, isOutput=False)
    wq_d = nc.declare_dram_parameter("wq", [D, HHD], MTD, isOutput=False)
    wk_d = nc.declare_dram_parameter("wk", [D, HHD], MTD, isOutput=False)
    wv_d = nc.declare_dram_parameter("wv", [D, HHD], MTD, isOutput=False)
    wo_d = nc.declare_dram_parameter("wo", [HHD, D], MTD, isOutput=False)
    bq_d = nc.declare_dram_parameter("bq", [1, HHD], MTD, isOutput=False)
    bk_d = nc.declare_dram_parameter("bk", [1, HHD], MTD, isOutput=False)
    bv_d = nc.declare_dram_parameter("bv", [1, HHD], MTD, isOutput=False)
    sel_d = nc.declare_dram_parameter("sel", [64, 128], F32, isOutput=False)
    ones_d = nc.declare_dram_parameter("ones1", [1, 512], F32, isOutput=False)
    zr_d = nc.declare_dram_parameter("zr", [64, 512], F32, isOutput=False)
    out_d = nc.declare_dram_parameter("out", [SEQ, D], F32, isOutput=True)

    def mcast(ap):
        # view a dram param as the matmul dtype
        return ap if mode == "bf16" else ap.bitcast(F32R)

    with tile.TileContext(nc) as tc:
        from contextlib import ExitStack

        with ExitStack() as ctx:
            ctx.enter_context(nc.allow_low_precision(
                reason="bf16/f32r matmul operands; psum accumulation is fp32"
            ))
            const = ctx.enter_context(tc.tile_pool(name="const", bufs=1))
            ones_r = const.tile([1, 512], mt, tag="ones")
            if mode == "bf16":
                nc.vector.memset(ones_r[:], 1.0)
            else:
                nc.sync.dma_start(ones_r[:], ones_d[:].bitcast(F32R))
            dones = const.tile([128, 1], BF16, tag="dones")
            nc.vector.memset(dones[:], 1.0)
            sel_sb = const.tile([64, 128], F32R, tag="sel")
            nc.sync.dma_start(sel_sb[:], sel_d[:].bitcast(F32R))
            ds = const.tile([64, 512], F32R, tag="ds")
            nc.sync.dma_start(ds[:], zr_d[:].bitcast(F32R))
            bq_sb = const.tile([1, HHD], mt, tag="bq")
            nc.sync.dma_start(bq_sb[:], mcast(bq_d[:]))
            bk_sb = const.tile([1, HHD], mt, tag="bk")
            nc.sync.dma_start(bk_sb[:], mcast(bk_d[:]))
            bv_sb = const.tile([1, HHD], mt, tag="bv")
            nc.sync.dma_start(bv_sb[:], mcast(bv_d[:]))

            # persistent activations for the attention phase
            qT = const.tile([128, HB, SEQ], mt, tag="qT")      # [hhd%128, blk, q]
            kT = const.tile([128, HB, SEQ], mt, tag="kT")
            v_sb = const.tile([128, NKT, HHD], BF16, tag="v")  # [k%128, ktile, hhd]

            wpool = ctx.enter_context(tc.tile_pool(name="w", bufs=2))
            mm_ps = ctx.enter_context(
                tc.tile_pool(name="mmps", bufs=2, space="PSUM")
            )

            def load_w(dram, cols):
                t = wpool.tile([128, D // 128, cols], mt, tag="w", bufs=2)
                nc.sync.dma_start(
                    t[:], mcast(dram.rearrange("(c p) n -> p c n", p=128))
                )
                return t

            # ---------------- phase 1: projections ----------------
            with tc.tile_pool(name="x", bufs=8) as xpool:

                def load_x(dram):
                    xs = []
                    for c in range(DCH):
                        t = xpool.tile([128, SEQ], mt, tag="xc", bufs=8)
                        nc.sync.dma_start(
                            t[:], mcast(dram[c * 128:(c + 1) * 128, :])
                        )
                        xs.append(t)
                    return xs

                def proj_T(dst, w_sb, b_sb, xs):
                    # dst[:, hb, q] = (x @ W + b)^T rows hb*128..+128
                    for hb in range(HB):
                        for (qo, cw) in QC:
                            ps = mm_ps.tile([128, 512], F32, tag="mm", bufs=2)
                            for c in range(DCH):
                                nc.tensor.matmul(
                                    ps[:, :cw],
                                    w_sb[:, c, hb * 128:(hb + 1) * 128],
                                    xs[c][:, qo:qo + cw],
                                    start=(c == 0), stop=False,
                                )
                            nc.tensor.matmul(
                                ps[:, :cw],
                                b_sb[0:1, hb * 128:(hb + 1) * 128],
                                ones_r[0:1, :cw],
                                start=False, stop=True,
                            )
                            nc.vector.tensor_copy(
                                dst[:, hb, qo:qo + cw], ps[:, :cw]
                            )

                def proj_v(dst, w_sb, b_sb, xs):
                    # dst[:, kt, hhd] = (x @ W + b) rows kt*128..
                    for kt, (ko, kh) in enumerate(KT):
                        ps = mm_ps.tile([128, 512], F32, tag="mm", bufs=2)
                        for c in range(DCH):
                            nc.tensor.matmul(
                                ps[:kh, :],
                                xs[c][:, ko:ko + kh],
                                w_sb[:, c, :],
                                start=(c == 0), stop=False,
                            )
                        nc.tensor.matmul(
                            ps[:kh, :],
                            ones_r[0:1, :kh],
                            b_sb[0:1, :],
                            start=False, stop=True,
                        )
                        nc.vector.tensor_copy(dst[:kh, kt, :], ps[:kh, :])

                wk_sb = load_w(wk_d, HHD)
                wv_sb = load_w(wv_d, HHD)
                xkv = load_x(xkvT)
                proj_T(kT, wk_sb, bk_sb, xkv)
                proj_v(v_sb, wv_sb, bv_sb, xkv)
                wq_sb = load_w(wq_d, HHD)
                xq = load_x(xqT)
                proj_T(qT, wq_sb, bq_sb, xq)

            wo_sb = wpool.tile([128, HB, D], mt, tag="w", bufs=2)
            nc.sync.dma_start(
                wo_sb[:], mcast(wo_d.rearrange("(c p) n -> p c n", p=128))
            )

            # ---------------- phase 2+3: attention + out-proj ----------------
            st_ps = ctx.enter_context(tc.tile_pool(name="stps", bufs=2, space="PSUM"))
            pair_ps = ctx.enter_context(tc.tile_pool(name="pairps", bufs=1, space="PSUM"))
            den_ps = ctx.enter_context(tc.tile_pool(name="denps", bufs=1, space="PSUM"))
            p_pool = ctx.enter_context(tc.tile_pool(name="p", bufs=4))
            an_pool = ctx.enter_context(tc.tile_pool(name="an", bufs=8))
            small = ctx.enter_context(tc.tile_pool(name="small", bufs=4))

            for (qo, cw) in QC:
                anorms = []
                for j in range(NPAIR):
                    den = den_ps.tile([128, 512], F32, tag="den", bufs=1)
                    pair = pair_ps.tile([128, 512], F32, tag="pair", bufs=1)
                    for kt, (ko, kh) in enumerate(KT):
                        st = st_ps.tile([128, 2, 512], F32, tag="st", bufs=2)
                        # S^T for the even/odd head of pair j (row-packed)
                        nc.tensor.matmul(
                            st[:kh, 0, :cw],
                            kT[0:64, j, ko:ko + kh],
                            qT[0:64, j, qo:qo + cw],
                            start=True, stop=True,
                        )
                        nc.tensor.matmul(
                            st[:kh, 1, :cw],
                            kT[64:128, j, ko:ko + kh],
                            qT[64:128, j, qo:qo + cw],
                            start=True, stop=True,
                        )
                        p = p_pool.tile([128, 2, 512], BF16, tag="p", bufs=4)
                        nc.scalar.activation(
                            p[:kh, :, :cw], st[:kh, :, :cw], AF.Exp,
                            scale=0.125,
                        )
                        # attn @ V (bf16, col-packed pair into one psum tile)
                        nc.tensor.matmul(
                            pair[0:64, :cw],
                            v_sb[0:kh, kt, (2 * j) * 64:(2 * j) * 64 + 64],
                            p[0:kh, 0, :cw],
                            start=(kt == 0), stop=(kt == NKT - 1),
                            skip_group_check=True,
                        )
                        nc.tensor.matmul(
                            pair[64:128, :cw],
                            v_sb[0:kh, kt, (2 * j + 1) * 64:(2 * j + 1) * 64 + 64],
                            p[0:kh, 1, :cw],
                            start=(kt == 0), stop=(kt == NKT - 1),
                            skip_group_check=True,
                        )
                        # softmax denominators (M=1 matmuls, col-packed)
                        nc.tensor.matmul(
                            den[0:1, :cw],
                            dones[0:kh, 0:1],
                            p[0:kh, 0, :cw],
                            start=(kt == 0), stop=(kt == NKT - 1),
                            tile_position=(0, 0), skip_group_check=True,
                        )
                        nc.tensor.matmul(
                            den[32:33, :cw],
                            dones[0:kh, 0:1],
                            p[0:kh, 1, :cw],
                            start=(kt == 0), stop=(kt == NKT - 1),
                            tile_position=(0, 32), skip_group_check=True,
                        )

                    # normalize: broadcast the 2 denominator rows across
                    # partitions via a tiny selector matmul, then one
                    # full-width approximate reciprocal
                    nc.vector.tensor_copy(ds[0:1, :cw], den[0:1, :cw])
                    nc.vector.tensor_copy(ds[32:33, :cw], den[32:33, :cw])
                    rb_ps = mm_ps.tile([128, 512], F32, tag="mm", bufs=2)
                    nc.tensor.matmul(
                        rb_ps[:, :cw],
                        sel_sb[0:64, 0:128],
                        ds[0:64, :cw],
                        start=True, stop=True,
                    )
                    rb_sb = small.tile([128, 512], F32, tag="rb", bufs=2)
                    nc.vector.reciprocal_approx_fast(rb_sb[:, :cw], rb_ps[:, :cw])
                    an = an_pool.tile([128, 512], mt, tag="an", bufs=8)
                    nc.vector.tensor_mul(
                        an[:, :cw], pair[:, :cw], rb_sb[:, :cw]
                    )
                    anorms.append(an)

                # out-projection for this q chunk (natural [q, d] layout)
                nsub = (cw + 127) // 128
                for s in range(nsub):
                    sw = min(128, cw - s * 128)
                    for dc in range(2):
                        op = mm_ps.tile([128, 512], F32, tag="mm", bufs=2)
                        for j in range(NPAIR):
                            nc.tensor.matmul(
                                op[:sw, :],
                                anorms[j][:, s * 128:s * 128 + sw],
                                wo_sb[:, j, dc * 512:(dc + 1) * 512],
                                start=(j == 0), stop=(j == NPAIR - 1),
                            )
                        osb = small.tile([128, 512], F32, tag="os", bufs=3)
                        nc.vector.tensor_copy(osb[:sw, :], op[:sw, :])
                        nc.sync.dma_start(
                            out_d[qo + s * 128:qo + s * 128 + sw,
                                  dc * 512:(dc + 1) * 512],
                            osb[:sw, :],
                        )

    nc.compile()
    return nc


_NC = {}


def _get_nc(mode=MODE):
    if mode not in _NC:
        _NC[mode] = _build(mode)
    return _NC[mode]


def _sel_const():
    # sel[r, m] routes reciprocal row r to output partitions m:
    # row 0 (even head) -> partitions 0..63, row 32 (odd) -> 64..127
    sel = np.zeros((64, 128), np.float32)
    sel[0, 0:64] = 1.0
    sel[32, 64:128] = 1.0
    return sel


def _shard_inputs(mode, inputs_q, inputs_kv, Wq, bq, Wk, bk, Wv, bv, Wo, bo):
    ndt = ml_dtypes.bfloat16 if mode == "bf16" else np.float32
    sel = _sel_const()
    ones1 = np.ones((1, 512), np.float32)
    zr = np.zeros((64, 512), np.float32)
    in_maps = []
    for b in range(B):
        xqT = np.ascontiguousarray(inputs_q[b].T).astype(ndt)
        xkvT = np.ascontiguousarray(inputs_kv[b].T).astype(ndt)
        for g in range(2):
            hs = slice(g * HG, (g + 1) * HG)
            in_maps.append({
                "xqT": xqT,
                "xkvT": xkvT,
                "wq": np.ascontiguousarray(Wq[:, hs, :].reshape(D, HHD)).astype(ndt),
                "wk": np.ascontiguousarray(Wk[:, hs, :].reshape(D, HHD)).astype(ndt),
                "wv": np.ascontiguousarray(Wv[:, hs, :].reshape(D, HHD)).astype(ndt),
                "wo": np.ascontiguousarray(Wo[hs].reshape(HHD, D)).astype(ndt),
                "bq": np.ascontiguousarray(bq[hs].reshape(1, HHD)).astype(ndt),
                "bk": np.ascontiguousarray(bk[hs].reshape(1, HHD)).astype(ndt),
                "bv": np.ascontiguousarray(bv[hs].reshape(1, HHD)).astype(ndt),
                "sel": sel,
                "ones1": ones1,
                "zr": zr,
            })
    return in_maps


def _run(inputs, trace=False, trace_kwargs=None, mode=MODE):
    nc = _get_nc(mode)
    in_maps = _shard_inputs(mode, **{k: np.asarray(v) for k, v in inputs.items()})
    res = run_bass_kernel_spmd(
        nc, in_maps, core_ids=list(range(2 * B)), trace=trace,
        **(trace_kwargs or {}),
    )
    bo = np.asarray(inputs["bo"], np.float32)
    out = np.empty((B, SEQ, D), np.float32)
    for b in range(B):
        out[b] = res.results[2 * b]["out"] + res.results[2 * b + 1]["out"] + bo
    return out, res


def kernel(**inputs):
    out, _ = _run(inputs, trace=False)
    return out


# revision 14
# speedup vs baseline: 1.3943x; 1.2319x over previous
"""Trainium2 Bass kernel for flax MultiHeadDotProductAttention.

Shapes (hardcoded): B=4, Q=K=1500, D=1024, H=16, HD=64.
Sharding: 8 cores = 4 batches x 2 head-groups (8 heads each).
Each core computes its batch's attention output for its 8 heads plus the
output projection restricted to those heads; the host sums the two
head-group partials per batch and adds bo.

Dataflow per core (all layouts chosen so no on-device transposes are
needed; host passes x pre-transposed):
  qT/kT [hhd, seq] and v [seq, hhd] via projection matmuls;
  S^T[k,q] = kT.T-slices @ qT (K=64, row-packed 2 heads per PE slot);
  P^T = exp(S^T/8) on ScalarE (psum->sbuf, bf16);
  attn_outT += v_tile.T @ P^T (bf16, col-packed 2 heads per slot) and
  denominators via ones-vector matmuls (4 heads col-packed per slot),
  two head-pairs interleaved per k step so PE has independent work
  while ScalarE exponentiates; normalization via a selector matmul
  broadcast + one full-width approximate reciprocal; out-projection
  consumes the normalized [hhd, q] tiles as stationary operands ->
  natural [q, d] output tiles DMA'd straight to HBM.

MODE: "bf16" (default) runs all big matmuls in bf16 (weight loads
overlap in-flight matmuls); "mixed" keeps projections/S^T/out-proj in
fp32r (higher precision, but each matmul pays a serialized weight load).
"""

import os
import sys

sys.path.insert(0, "/opt/trn_rl_repo")

import numpy as np  # noqa: E402
import ml_dtypes  # noqa: E402
import concourse.bacc as bacc  # noqa: E402
import concourse.mybir as mybir  # noqa: E402
import concourse.tile as tile  # noqa: E402
from concourse.bass_utils import run_bass_kernel_spmd  # noqa: E402

F32 = mybir.dt.float32
F32R = mybir.dt.float32r
BF16 = mybir.dt.bfloat16
AF = mybir.ActivationFunctionType

B, SEQ, D, H, HD = 4, 1500, 1024, 16, 64
HG = 8                      # heads per group
HHD = HG * HD               # 512
DCH = D // 128              # 8 d-chunks
HB = HHD // 128             # 4 hhd blocks (2 heads each)
NPAIR = HB                  # 4 head pairs per group
QC = [(0, 512), (512, 512), (1024, 476)]          # q chunks
KT = [(i * 128, min(128, SEQ - i * 128)) for i in range((SEQ + 127) // 128)]
NKT = len(KT)               # 12 (last tile 92 rows)

MODE = os.environ.get("BASS_MM_DTYPE", "bf16")


def _build(mode, with_bias):
    mt = BF16 if mode == "bf16" else F32R          # big-matmul operand dtype
    MTD = BF16 if mode == "bf16" else F32          # dram dtype for x/w/b

    nc = bacc.Bacc("TRN2", target_bir_lowering=False, debug=False, num_devices=8)

    xqT = nc.declare_dram_parameter("xqT", [D, SEQ], MTD, isOutput=False)
    xkvT = nc.declare_dram_parameter("xkvT", [D, SEQ], MTD, isOutput=False)
    wq_d = nc.declare_dram_parameter("wq", [D, HHD], MTD, isOutput=False)
    wk_d = nc.declare_dram_parameter("wk", [D, HHD], MTD, isOutput=False)
    wv_d = nc.declare_dram_parameter("wv", [D, HHD], MTD, isOutput=False)
    wo_d = nc.declare_dram_parameter("wo", [HHD, D], MTD, isOutput=False)
    bq_d = nc.declare_dram_parameter("bq", [1, HHD], MTD, isOutput=False)
    bk_d = nc.declare_dram_parameter("bk", [1, HHD], MTD, isOutput=False)
    bv_d = nc.declare_dram_parameter("bv", [1, HHD], MTD, isOutput=False)
    sel_d = nc.declare_dram_parameter("sel", [128, 256], F32, isOutput=False)
    ones_d = nc.declare_dram_parameter("ones1", [1, 512], F32, isOutput=False)
    zr_d = nc.declare_dram_parameter("zr", [128, 512], F32, isOutput=False)
    out_d = nc.declare_dram_parameter("out", [SEQ, D], F32, isOutput=True)

    def mcast(ap):
        # view a dram param as the matmul dtype
        return ap if mode == "bf16" else ap.bitcast(F32R)

    with tile.TileContext(nc) as tc:
        from contextlib import ExitStack

        with ExitStack() as ctx:
            ctx.enter_context(nc.allow_low_precision(
                reason="bf16/f32r matmul operands; psum accumulation is fp32"
            ))
            const = ctx.enter_context(tc.tile_pool(name="const", bufs=1))
            ones_r = const.tile([1, 512], mt, tag="ones")
            if mode == "bf16":
                nc.vector.memset(ones_r[:], 1.0)
            else:
                nc.sync.dma_start(ones_r[:], ones_d[:].bitcast(F32R))
            dones = const.tile([128, 1], BF16, tag="dones")
            nc.vector.memset(dones[:], 1.0)
            sel_sb = const.tile([128, 256], F32R, tag="sel")
            nc.sync.dma_start(sel_sb[:], sel_d[:].bitcast(F32R))
            ds = const.tile([128, 512], F32R, tag="ds")
            nc.sync.dma_start(ds[:], zr_d[:].bitcast(F32R))
            bq_sb = const.tile([1, HHD], mt, tag="bq")
            nc.sync.dma_start(bq_sb[:], mcast(bq_d[:]))
            bk_sb = const.tile([1, HHD], mt, tag="bk")
            nc.sync.dma_start(bk_sb[:], mcast(bk_d[:]))
            bv_sb = const.tile([1, HHD], mt, tag="bv")
            nc.sync.dma_start(bv_sb[:], mcast(bv_d[:]))

            # persistent activations for the attention phase
            qT = const.tile([128, HB, SEQ], mt, tag="qT")      # [hhd%128, blk, q]
            kT = const.tile([128, HB, SEQ], mt, tag="kT")
            v_sb = const.tile([128, NKT, HHD], BF16, tag="v")  # [k%128, ktile, hhd]

            wpool = ctx.enter_context(tc.tile_pool(name="w", bufs=2))

            def load_w(dram, cols):
                t = wpool.tile([128, D // 128, cols], mt, tag="w", bufs=2)
                nc.sync.dma_start(
                    t[:], mcast(dram.rearrange("(c p) n -> p c n", p=128))
                )
                return t

            # ---------------- phase 1: projections ----------------
            with tc.tile_pool(name="x", bufs=8) as xpool, \
                 tc.tile_pool(name="mmps", bufs=2, space="PSUM") as mm_ps:

                def load_x(dram):
                    xs = []
                    for c in range(DCH):
                        t = xpool.tile([128, SEQ], mt, tag="xc", bufs=8)
                        nc.sync.dma_start(
                            t[:], mcast(dram[c * 128:(c + 1) * 128, :])
                        )
                        xs.append(t)
                    return xs

                def proj_T(dst, w_sb, b_sb, xs):
                    # dst[:, hb, q] = (x @ W + b)^T rows hb*128..+128
                    for hb in range(HB):
                        for (qo, cw) in QC:
                            ps = mm_ps.tile([128, 512], F32, tag="mm", bufs=2)
                            for c in range(DCH):
                                nc.tensor.matmul(
                                    ps[:, :cw],
                                    w_sb[:, c, hb * 128:(hb + 1) * 128],
                                    xs[c][:, qo:qo + cw],
                                    start=(c == 0), stop=(not with_bias and c == DCH - 1),
                                )
                            if with_bias:
                                nc.tensor.matmul(
                                    ps[:, :cw],
                                    b_sb[0:1, hb * 128:(hb + 1) * 128],
                                    ones_r[0:1, :cw],
                                    start=False, stop=True,
                                )
                            nc.vector.tensor_copy(
                                dst[:, hb, qo:qo + cw], ps[:, :cw]
                            )

                def proj_v(dst, w_sb, b_sb, xs):
                    # dst[:, kt, hhd] = (x @ W + b) rows kt*128..
                    for kt, (ko, kh) in enumerate(KT):
                        ps = mm_ps.tile([128, 512], F32, tag="mm", bufs=2)
                        for c in range(DCH):
                            nc.tensor.matmul(
                                ps[:kh, :],
                                xs[c][:, ko:ko + kh],
                                w_sb[:, c, :],
                                start=(c == 0), stop=(not with_bias and c == DCH - 1),
                            )
                        if with_bias:
                            nc.tensor.matmul(
                                ps[:kh, :],
                                ones_r[0:1, :kh],
                                b_sb[0:1, :],
                                start=False, stop=True,
                            )
                        nc.vector.tensor_copy(dst[:kh, kt, :], ps[:kh, :])

                wk_sb = load_w(wk_d, HHD)
                wv_sb = load_w(wv_d, HHD)
                xkv = load_x(xkvT)
                proj_T(kT, wk_sb, bk_sb, xkv)
                proj_v(v_sb, wv_sb, bv_sb, xkv)
                wq_sb = load_w(wq_d, HHD)
                xq = load_x(xqT)
                proj_T(qT, wq_sb, bq_sb, xq)

            wo_sb = wpool.tile([128, HB, D], mt, tag="w", bufs=2)
            nc.sync.dma_start(
                wo_sb[:], mcast(wo_d.rearrange("(c p) n -> p c n", p=128))
            )

            # ---------------- phase 2+3: attention + out-proj ----------------
            st_ps = ctx.enter_context(tc.tile_pool(name="stps", bufs=2, space="PSUM"))
            pair_ps = ctx.enter_context(tc.tile_pool(name="pairps", bufs=2, space="PSUM"))
            den_ps = ctx.enter_context(tc.tile_pool(name="denps", bufs=1, space="PSUM"))
            rbo_ps = ctx.enter_context(tc.tile_pool(name="rbops", bufs=1, space="PSUM"))
            p_pool = ctx.enter_context(tc.tile_pool(name="p", bufs=6))
            an_pool = ctx.enter_context(tc.tile_pool(name="an", bufs=8))
            small = ctx.enter_context(tc.tile_pool(name="small", bufs=4))

            for (qo, cw) in QC:
                anorms = []
                for jp in range(2):  # two groups of two head-pairs
                    den = den_ps.tile([128, 512], F32, tag="den", bufs=1)
                    pairs = [
                        pair_ps.tile([128, 512], F32, tag="pair", bufs=2,
                                     name=f"pair_{jp}_{g2}")
                        for g2 in range(2)
                    ]
                    for kt, (ko, kh) in enumerate(KT):
                        for g in range(2):  # interleave the two pairs
                            j = jp * 2 + g
                            pr = pairs[g]
                            st = st_ps.tile([128, 2, 512], F32, tag="st", bufs=2)
                            # S^T for the even/odd head of pair j (row-packed)
                            nc.tensor.matmul(
                                st[:kh, 0, :cw],
                                kT[0:64, j, ko:ko + kh],
                                qT[0:64, j, qo:qo + cw],
                                start=True, stop=True,
                            )
                            nc.tensor.matmul(
                                st[:kh, 1, :cw],
                                kT[64:128, j, ko:ko + kh],
                                qT[64:128, j, qo:qo + cw],
                                start=True, stop=True,
                            )
                            p = p_pool.tile([128, 2, 512], BF16, tag="p", bufs=6)
                            nc.scalar.activation(
                                p[:kh, :, :cw], st[:kh, :, :cw], AF.Exp,
                                scale=0.125,
                            )
                            # attn @ V (bf16, col-packed pair in one psum tile)
                            nc.tensor.matmul(
                                pr[0:64, :cw],
                                v_sb[0:kh, kt, (2 * j) * 64:(2 * j) * 64 + 64],
                                p[0:kh, 0, :cw],
                                start=(kt == 0), stop=(kt == NKT - 1),
                                skip_group_check=True,
                            )
                            nc.tensor.matmul(
                                pr[64:128, :cw],
                                v_sb[0:kh, kt, (2 * j + 1) * 64:(2 * j + 1) * 64 + 64],
                                p[0:kh, 1, :cw],
                                start=(kt == 0), stop=(kt == NKT - 1),
                                skip_group_check=True,
                            )
                            # softmax denominators: M=1 matmuls col-packed
                            # four per den bank (rows 0,32 pair g=0; 64,96 g=1)
                            nc.tensor.matmul(
                                den[64 * g:64 * g + 1, :cw],
                                dones[0:kh, 0:1],
                                p[0:kh, 0, :cw],
                                start=(kt == 0), stop=(kt == NKT - 1),
                                tile_position=(0, 64 * g), skip_group_check=True,
                            )
                            nc.tensor.matmul(
                                den[64 * g + 32:64 * g + 33, :cw],
                                dones[0:kh, 0:1],
                                p[0:kh, 1, :cw],
                                start=(kt == 0), stop=(kt == NKT - 1),
                                tile_position=(0, 64 * g + 32), skip_group_check=True,
                            )

                    # normalize both pairs: copy den rows beside their sel
                    # rows, selector-matmul broadcast, approx reciprocal,
                    # then fold into the psum->sbuf copy
                    for g in range(2):
                        nc.vector.tensor_copy(
                            ds[64 * g:64 * g + 1, :cw], den[64 * g:64 * g + 1, :cw]
                        )
                        nc.vector.tensor_copy(
                            ds[64 * g + 32:64 * g + 33, :cw],
                            den[64 * g + 32:64 * g + 33, :cw],
                        )
                    for g in range(2):
                        rb_ps = rbo_ps.tile([128, 512], F32, tag="rbo", bufs=1)
                        nc.tensor.matmul(
                            rb_ps[:, :cw],
                            sel_sb[:, g * 128:(g + 1) * 128],
                            ds[:, :cw],
                            start=True, stop=True,
                        )
                        rb_sb = small.tile([128, 512], F32, tag="rb", bufs=2)
                        nc.vector.reciprocal_approx_fast(rb_sb[:, :cw], rb_ps[:, :cw])
                        an = an_pool.tile([128, 512], mt, tag="an", bufs=8)
                        nc.vector.tensor_mul(
                            an[:, :cw], pairs[g][:, :cw], rb_sb[:, :cw]
                        )
                        anorms.append(an)

                # out-projection for this q chunk (natural [q, d] layout)
                nsub = (cw + 127) // 128
                for s in range(nsub):
                    sw = min(128, cw - s * 128)
                    for dc in range(2):
                        op = rbo_ps.tile([128, 512], F32, tag="rbo", bufs=1)
                        for j in range(NPAIR):
                            nc.tensor.matmul(
                                op[:sw, :],
                                anorms[j][:, s * 128:s * 128 + sw],
                                wo_sb[:, j, dc * 512:(dc + 1) * 512],
                                start=(j == 0), stop=(j == NPAIR - 1),
                            )
                        osb = small.tile([128, 512], F32, tag="os", bufs=3)
                        nc.vector.tensor_copy(osb[:sw, :], op[:sw, :])
                        nc.sync.dma_start(
                            out_d[qo + s * 128:qo + s * 128 + sw,
                                  dc * 512:(dc + 1) * 512],
                            osb[:sw, :],
                        )

    nc.compile()
    return nc


_NC = {}


def _get_nc(mode=MODE, with_bias=False):
    key = (mode, with_bias)
    if key not in _NC:
        _NC[key] = _build(mode, with_bias)
    return _NC[key]


def _sel_const():
    # sel[r, g*128 + m] routes den row r to output partitions m for pair
    # g of the group: pair 0 uses den rows 0 (->parts 0..63) and 32
    # (->64..127); pair 1 uses rows 64 and 96.
    sel = np.zeros((128, 256), np.float32)
    sel[0, 0:64] = 1.0
    sel[32, 64:128] = 1.0
    sel[64, 128:192] = 1.0
    sel[96, 192:256] = 1.0
    return sel


def _shard_inputs(mode, inputs_q, inputs_kv, Wq, bq, Wk, bk, Wv, bv, Wo, bo):
    ndt = ml_dtypes.bfloat16 if mode == "bf16" else np.float32
    sel = _sel_const()
    ones1 = np.ones((1, 512), np.float32)
    zr = np.zeros((128, 512), np.float32)
    in_maps = []
    for b in range(B):
        xqT = np.ascontiguousarray(inputs_q[b].T).astype(ndt)
        xkvT = np.ascontiguousarray(inputs_kv[b].T).astype(ndt)
        for g in range(2):
            hs = slice(g * HG, (g + 1) * HG)
            in_maps.append({
                "xqT": xqT,
                "xkvT": xkvT,
                "wq": np.ascontiguousarray(Wq[:, hs, :].reshape(D, HHD)).astype(ndt),
                "wk": np.ascontiguousarray(Wk[:, hs, :].reshape(D, HHD)).astype(ndt),
                "wv": np.ascontiguousarray(Wv[:, hs, :].reshape(D, HHD)).astype(ndt),
                "wo": np.ascontiguousarray(Wo[hs].reshape(HHD, D)).astype(ndt),
                "bq": np.ascontiguousarray(bq[hs].reshape(1, HHD)).astype(ndt),
                "bk": np.ascontiguousarray(bk[hs].reshape(1, HHD)).astype(ndt),
                "bv": np.ascontiguousarray(bv[hs].reshape(1, HHD)).astype(ndt),
                "sel": sel,
                "ones1": ones1,
                "zr": zr,
            })
    return in_maps


def _run(inputs, trace=False, trace_kwargs=None, mode=MODE):
    inputs = {k: np.asarray(v) for k, v in inputs.items()}
    with_bias = bool(
        np.any(inputs["bq"]) or np.any(inputs["bk"]) or np.any(inputs["bv"])
    )
    nc = _get_nc(mode, with_bias)
    in_maps = _shard_inputs(mode, **inputs)
    res = run_bass_kernel_spmd(
        nc, in_maps, core_ids=list(range(2 * B)), trace=trace,
        **(trace_kwargs or {}),
    )
    bo = np.asarray(inputs["bo"], np.float32)
    out = np.empty((B, SEQ, D), np.float32)
    for b in range(B):
        out[b] = res.results[2 * b]["out"] + res.results[2 * b + 1]["out"] + bo
    return out, res


def kernel(**inputs):
    out, _ = _run(inputs, trace=False)
    return out


# revision 15
# speedup vs baseline: 1.7358x; 1.2449x over previous
"""Trainium2 Bass kernel for flax MultiHeadDotProductAttention.

Shapes (hardcoded): B=4, Q=K=1500, D=1024, H=16, HD=64.
Sharding: 8 cores = 4 batches x 2 head-groups (8 heads each).
Each core computes its batch's attention output for its 8 heads plus the
output projection restricted to those heads; the host sums the two
head-group partials per batch and adds bo.

Dataflow per core (all layouts chosen so no on-device transposes are
needed; host passes x pre-transposed):
  qT/kT [hhd, seq] and v [seq, hhd] via projection matmuls;
  S^T[k,q] = kT.T-slices @ qT (K=64, row-packed 2 heads per PE slot);
  P^T = exp(S^T/8) on ScalarE (psum->sbuf, bf16);
  attn_outT += v_tile.T @ P^T (bf16, col-packed 2 heads per slot) and
  denominators via ones-vector matmuls (4 heads col-packed per slot),
  two head-pairs interleaved per k step so PE has independent work
  while ScalarE exponentiates; normalization via a selector matmul
  broadcast + one full-width approximate reciprocal; out-projection
  consumes the normalized [hhd, q] tiles as stationary operands ->
  natural [q, d] output tiles DMA'd straight to HBM.

MODE: "bf16" (default) runs all big matmuls in bf16 (weight loads
overlap in-flight matmuls); "mixed" keeps projections/S^T/out-proj in
fp32r (higher precision, but each matmul pays a serialized weight load).
"""

import os
import sys

sys.path.insert(0, "/opt/trn_rl_repo")

import numpy as np  # noqa: E402
import ml_dtypes  # noqa: E402
import concourse.bacc as bacc  # noqa: E402
import concourse.mybir as mybir  # noqa: E402
import concourse.tile as tile  # noqa: E402
from concourse.bass_utils import run_bass_kernel_spmd  # noqa: E402

F32 = mybir.dt.float32
F32R = mybir.dt.float32r
BF16 = mybir.dt.bfloat16
AF = mybir.ActivationFunctionType

B, SEQ, D, H, HD = 4, 1500, 1024, 16, 64
HG = 8                      # heads per group
HHD = HG * HD               # 512
DCH = D // 128              # 8 d-chunks
HB = HHD // 128             # 4 hhd blocks (2 heads each)
NPAIR = HB                  # 4 head pairs per group
QC = [(0, 512), (512, 512), (1024, 476)]          # q chunks
KT = [(i * 128, min(128, SEQ - i * 128)) for i in range((SEQ + 127) // 128)]
NKT = len(KT)               # 12 (last tile 92 rows)

MODE = os.environ.get("BASS_MM_DTYPE", "bf16")


def _build(mode, with_bias):
    mt = BF16 if mode == "bf16" else F32R          # big-matmul operand dtype
    MTD = BF16 if mode == "bf16" else F32          # dram dtype for x/w/b

    nc = bacc.Bacc("TRN2", target_bir_lowering=False, debug=False, num_devices=8)

    xqT = nc.declare_dram_parameter("xqT", [D, SEQ], MTD, isOutput=False)
    xkvT = nc.declare_dram_parameter("xkvT", [D, SEQ], MTD, isOutput=False)
    wq_d = nc.declare_dram_parameter("wq", [D, HHD], MTD, isOutput=False)
    wk_d = nc.declare_dram_parameter("wk", [D, HHD], MTD, isOutput=False)
    wv_d = nc.declare_dram_parameter("wv", [D, HHD], MTD, isOutput=False)
    wo_d = nc.declare_dram_parameter("wo", [HHD, D], MTD, isOutput=False)
    bq_d = nc.declare_dram_parameter("bq", [1, HHD], MTD, isOutput=False)
    bk_d = nc.declare_dram_parameter("bk", [1, HHD], MTD, isOutput=False)
    bv_d = nc.declare_dram_parameter("bv", [1, HHD], MTD, isOutput=False)
    sel_d = nc.declare_dram_parameter("sel", [128, 256], F32, isOutput=False)
    ones_d = nc.declare_dram_parameter("ones1", [1, 512], F32, isOutput=False)
    zr_d = nc.declare_dram_parameter("zr", [128, 512], F32, isOutput=False)
    out_d = nc.declare_dram_parameter("out", [SEQ, D], F32, isOutput=True)

    def mcast(ap):
        # view a dram param as the matmul dtype
        return ap if mode == "bf16" else ap.bitcast(F32R)

    with tile.TileContext(nc) as tc:
        from contextlib import ExitStack

        with ExitStack() as ctx:
            ctx.enter_context(nc.allow_low_precision(
                reason="bf16/f32r matmul operands; psum accumulation is fp32"
            ))
            const = ctx.enter_context(tc.tile_pool(name="const", bufs=1))
            ones_r = const.tile([1, 512], mt, tag="ones")
            if mode == "bf16":
                nc.vector.memset(ones_r[:], 1.0)
            else:
                nc.sync.dma_start(ones_r[:], ones_d[:].bitcast(F32R))
            dones = const.tile([128, 1], BF16, tag="dones")
            nc.vector.memset(dones[:], 1.0)
            sel_sb = const.tile([128, 256], F32R, tag="sel")
            nc.sync.dma_start(sel_sb[:], sel_d[:].bitcast(F32R))
            ds = const.tile([128, 512], F32R, tag="ds")
            nc.sync.dma_start(ds[:], zr_d[:].bitcast(F32R))
            bq_sb = const.tile([1, HHD], mt, tag="bq")
            nc.sync.dma_start(bq_sb[:], mcast(bq_d[:]))
            bk_sb = const.tile([1, HHD], mt, tag="bk")
            nc.sync.dma_start(bk_sb[:], mcast(bk_d[:]))
            bv_sb = const.tile([1, HHD], mt, tag="bv")
            nc.sync.dma_start(bv_sb[:], mcast(bv_d[:]))

            # persistent activations for the attention phase
            qT = const.tile([128, HB, SEQ], mt, tag="qT")      # [hhd%128, blk, q]
            kT = const.tile([128, HB, SEQ], mt, tag="kT")
            v_sb = const.tile([128, NKT, HHD], BF16, tag="v")  # [k%128, ktile, hhd]

            wpool = ctx.enter_context(tc.tile_pool(name="w", bufs=2))

            def load_w(dram, cols):
                t = wpool.tile([128, D // 128, cols], mt, tag="w", bufs=2)
                nc.sync.dma_start(
                    t[:], mcast(dram.rearrange("(c p) n -> p c n", p=128))
                )
                return t

            # ---------------- phase 1: projections ----------------
            with tc.tile_pool(name="x", bufs=8) as xpool, \
                 tc.tile_pool(name="mmps", bufs=2, space="PSUM") as mm_ps:

                def load_x(dram):
                    xs = []
                    for c in range(DCH):
                        t = xpool.tile([128, SEQ], mt, tag="xc", bufs=8)
                        nc.sync.dma_start(
                            t[:], mcast(dram[c * 128:(c + 1) * 128, :])
                        )
                        xs.append(t)
                    return xs

                def proj_T(dst, w_sb, b_sb, xs):
                    # dst[:, hb, q] = (x @ W + b)^T rows hb*128..+128
                    for hb in range(HB):
                        for (qo, cw) in QC:
                            ps = mm_ps.tile([128, 512], F32, tag="mm", bufs=2)
                            for c in range(DCH):
                                nc.tensor.matmul(
                                    ps[:, :cw],
                                    w_sb[:, c, hb * 128:(hb + 1) * 128],
                                    xs[c][:, qo:qo + cw],
                                    start=(c == 0), stop=(not with_bias and c == DCH - 1),
                                )
                            if with_bias:
                                nc.tensor.matmul(
                                    ps[:, :cw],
                                    b_sb[0:1, hb * 128:(hb + 1) * 128],
                                    ones_r[0:1, :cw],
                                    start=False, stop=True,
                                )
                            nc.vector.tensor_copy(
                                dst[:, hb, qo:qo + cw], ps[:, :cw]
                            )

                def proj_v(dst, w_sb, b_sb, xs):
                    # dst[:, kt, hhd] = (x @ W + b) rows kt*128..
                    for kt, (ko, kh) in enumerate(KT):
                        ps = mm_ps.tile([128, 512], F32, tag="mm", bufs=2)
                        for c in range(DCH):
                            nc.tensor.matmul(
                                ps[:kh, :],
                                xs[c][:, ko:ko + kh],
                                w_sb[:, c, :],
                                start=(c == 0), stop=(not with_bias and c == DCH - 1),
                            )
                        if with_bias:
                            nc.tensor.matmul(
                                ps[:kh, :],
                                ones_r[0:1, :kh],
                                b_sb[0:1, :],
                                start=False, stop=True,
                            )
                        nc.vector.tensor_copy(dst[:kh, kt, :], ps[:kh, :])

                wk_sb = load_w(wk_d, HHD)
                wv_sb = load_w(wv_d, HHD)
                xkv = load_x(xkvT)
                proj_T(kT, wk_sb, bk_sb, xkv)
                proj_v(v_sb, wv_sb, bv_sb, xkv)
                wq_sb = load_w(wq_d, HHD)
                xq = load_x(xqT)
                proj_T(qT, wq_sb, bq_sb, xq)

            wo_sb = wpool.tile([128, HB, D], mt, tag="w", bufs=2)
            nc.sync.dma_start(
                wo_sb[:], mcast(wo_d.rearrange("(c p) n -> p c n", p=128))
            )

            # ---------------- phase 2+3: attention + out-proj ----------------
            st_ps = ctx.enter_context(tc.tile_pool(name="stps", bufs=2, space="PSUM"))
            pair_ps = ctx.enter_context(tc.tile_pool(name="pairps", bufs=2, space="PSUM"))
            den_ps = ctx.enter_context(tc.tile_pool(name="denps", bufs=1, space="PSUM"))
            rbo_ps = ctx.enter_context(tc.tile_pool(name="rbops", bufs=1, space="PSUM"))
            p_pool = ctx.enter_context(tc.tile_pool(name="p", bufs=6))
            an_pool = ctx.enter_context(tc.tile_pool(name="an", bufs=8))
            small = ctx.enter_context(tc.tile_pool(name="small", bufs=4))

            for (qo, cw) in QC:
                anorms = []
                for jp in range(2):  # two groups of two head-pairs
                    den = den_ps.tile([128, 512], F32, tag="den", bufs=1)
                    pairs = [
                        pair_ps.tile([128, 512], F32, tag="pair", bufs=2,
                                     name=f"pair_{jp}_{g2}")
                        for g2 in range(2)
                    ]
                    # software-pipelined: S^T/exp for step kt are emitted one
                    # step ahead of the attn@V/den consumers, so the in-order
                    # PE never waits on ScalarE's exp.
                    pend = [None, None]
                    for kt in range(NKT + 1):
                        if kt < NKT:
                            ko, kh = KT[kt]
                            for g in range(2):
                                j = jp * 2 + g
                                st = st_ps.tile([128, 2, 512], F32, tag="st",
                                                bufs=2, name=f"st_{g}")
                                nc.tensor.matmul(
                                    st[:kh, 0, :cw],
                                    kT[0:64, j, ko:ko + kh],
                                    qT[0:64, j, qo:qo + cw],
                                    start=True, stop=True,
                                )
                                nc.tensor.matmul(
                                    st[:kh, 1, :cw],
                                    kT[64:128, j, ko:ko + kh],
                                    qT[64:128, j, qo:qo + cw],
                                    start=True, stop=True,
                                )
                                p = p_pool.tile([128, 2, 512], BF16, tag="p",
                                                bufs=6, name=f"p_{g}")
                                nc.scalar.activation(
                                    p[:kh, :, :cw], st[:kh, :, :cw], AF.Exp,
                                    scale=0.125,
                                )
                                pend[g] = p
                        if kt > 0:
                            kc = kt - 1
                            ko, kh = KT[kc]
                            for g in range(2):
                                j = jp * 2 + g
                                pr = pairs[g]
                                p = pend2[g]
                                # attn @ V (bf16, col-packed in one psum tile)
                                nc.tensor.matmul(
                                    pr[0:64, :cw],
                                    v_sb[0:kh, kc, (2 * j) * 64:(2 * j) * 64 + 64],
                                    p[0:kh, 0, :cw],
                                    start=(kc == 0), stop=(kc == NKT - 1),
                                    skip_group_check=True,
                                )
                                nc.tensor.matmul(
                                    pr[64:128, :cw],
                                    v_sb[0:kh, kc, (2 * j + 1) * 64:(2 * j + 1) * 64 + 64],
                                    p[0:kh, 1, :cw],
                                    start=(kc == 0), stop=(kc == NKT - 1),
                                    skip_group_check=True,
                                )
                            for g in range(2):
                                # denominators: four M=1 matmuls col-packed
                                # into one bank (rows 0,32 pair 0; 64,96 pair 1)
                                p = pend2[g]
                                nc.tensor.matmul(
                                    den[64 * g:64 * g + 1, :cw],
                                    dones[0:kh, 0:1],
                                    p[0:kh, 0, :cw],
                                    start=(kc == 0), stop=(kc == NKT - 1),
                                    tile_position=(0, 64 * g), skip_group_check=True,
                                )
                                nc.tensor.matmul(
                                    den[64 * g + 32:64 * g + 33, :cw],
                                    dones[0:kh, 0:1],
                                    p[0:kh, 1, :cw],
                                    start=(kc == 0), stop=(kc == NKT - 1),
                                    tile_position=(0, 64 * g + 32), skip_group_check=True,
                                )
                        pend2 = list(pend)

                    # normalize both pairs: copy den rows beside their sel
                    # rows, selector-matmul broadcast, approx reciprocal,
                    # then fold into the psum->sbuf copy
                    for g in range(2):
                        nc.vector.tensor_copy(
                            ds[64 * g:64 * g + 1, :cw], den[64 * g:64 * g + 1, :cw]
                        )
                        nc.vector.tensor_copy(
                            ds[64 * g + 32:64 * g + 33, :cw],
                            den[64 * g + 32:64 * g + 33, :cw],
                        )
                    for g in range(2):
                        rb_ps = rbo_ps.tile([128, 512], F32, tag="rbo", bufs=1)
                        nc.tensor.matmul(
                            rb_ps[:, :cw],
                            sel_sb[:, g * 128:(g + 1) * 128],
                            ds[:, :cw],
                            start=True, stop=True,
                        )
                        rb_sb = small.tile([128, 512], F32, tag="rb", bufs=2)
                        nc.vector.reciprocal_approx_fast(rb_sb[:, :cw], rb_ps[:, :cw])
                        an = an_pool.tile([128, 512], mt, tag="an", bufs=8)
                        nc.vector.tensor_mul(
                            an[:, :cw], pairs[g][:, :cw], rb_sb[:, :cw]
                        )
                        anorms.append(an)

                # out-projection for this q chunk (natural [q, d] layout)
                nsub = (cw + 127) // 128
                for s in range(nsub):
                    sw = min(128, cw - s * 128)
                    for dc in range(2):
                        op = rbo_ps.tile([128, 512], F32, tag="rbo", bufs=1)
                        for j in range(NPAIR):
                            nc.tensor.matmul(
                                op[:sw, :],
                                anorms[j][:, s * 128:s * 128 + sw],
                                wo_sb[:, j, dc * 512:(dc + 1) * 512],
                                start=(j == 0), stop=(j == NPAIR - 1),
                            )
                        osb = small.tile([128, 512], F32, tag="os", bufs=3)
                        nc.vector.tensor_copy(osb[:sw, :], op[:sw, :])
                        nc.sync.dma_start(
                            out_d[qo + s * 128:qo + s * 128 + sw,
                                  dc * 512:(dc + 1) * 512],
                            osb[:sw, :],
                        )

    nc.compile()
    return nc


_NC = {}


def _get_nc(mode=MODE, with_bias=False):
    key = (mode, with_bias)
    if key not in _NC:
        _NC[key] = _build(mode, with_bias)
    return _NC[key]


def _sel_const():
    # sel[r, g*128 + m] routes den row r to output partitions m for pair
    # g of the group: pair 0 uses den rows 0 (->parts 0..63) and 32
    # (->64..127); pair 1 uses rows 64 and 96.
    sel = np.zeros((128, 256), np.float32)
    sel[0, 0:64] = 1.0
    sel[32, 64:128] = 1.0
    sel[64, 128:192] = 1.0
    sel[96, 192:256] = 1.0
    return sel


def _shard_inputs(mode, inputs_q, inputs_kv, Wq, bq, Wk, bk, Wv, bv, Wo, bo):
    ndt = ml_dtypes.bfloat16 if mode == "bf16" else np.float32
    sel = _sel_const()
    ones1 = np.ones((1, 512), np.float32)
    zr = np.zeros((128, 512), np.float32)
    in_maps = []
    for b in range(B):
        xqT = np.ascontiguousarray(inputs_q[b].T).astype(ndt)
        xkvT = np.ascontiguousarray(inputs_kv[b].T).astype(ndt)
        for g in range(2):
            hs = slice(g * HG, (g + 1) * HG)
            in_maps.append({
                "xqT": xqT,
                "xkvT": xkvT,
                "wq": np.ascontiguousarray(Wq[:, hs, :].reshape(D, HHD)).astype(ndt),
                "wk": np.ascontiguousarray(Wk[:, hs, :].reshape(D, HHD)).astype(ndt),
                "wv": np.ascontiguousarray(Wv[:, hs, :].reshape(D, HHD)).astype(ndt),
                "wo": np.ascontiguousarray(Wo[hs].reshape(HHD, D)).astype(ndt),
                "bq": np.ascontiguousarray(bq[hs].reshape(1, HHD)).astype(ndt),
                "bk": np.ascontiguousarray(bk[hs].reshape(1, HHD)).astype(ndt),
                "bv": np.ascontiguousarray(bv[hs].reshape(1, HHD)).astype(ndt),
                "sel": sel,
                "ones1": ones1,
                "zr": zr,
            })
    return in_maps


def _run(inputs, trace=False, trace_kwargs=None, mode=MODE):
    inputs = {k: np.asarray(v) for k, v in inputs.items()}
    with_bias = bool(
        np.any(inputs["bq"]) or np.any(inputs["bk"]) or np.any(inputs["bv"])
    )
    nc = _get_nc(mode, with_bias)
    in_maps = _shard_inputs(mode, **inputs)
    res = run_bass_kernel_spmd(
        nc, in_maps, core_ids=list(range(2 * B)), trace=trace,
        **(trace_kwargs or {}),
    )
    bo = np.asarray(inputs["bo"], np.float32)
    out = np.empty((B, SEQ, D), np.float32)
    for b in range(B):
        out[b] = res.results[2 * b]["out"] + res.results[2 * b + 1]["out"] + bo
    return out, res


def kernel(**inputs):
    out, _ = _run(inputs, trace=False)
    return out
